# revision 27
# baseline (speedup 1.0000x reference)
"""Multi-head attention (B=4, S=2048, D=768, H=12) on 8 TRN2 NeuronCores.

Sharding: core c = (batch b = c//2, query-half qh = c%2). Each core computes
the full 12-head attention for its 1024 query rows of its batch (K/V
projections duplicated across the pair of cores sharing a batch), and writes
disjoint slices of both outputs. No collectives.

Per-core kernel (all matmuls bf16 with f32 PSUM accumulate):
  - host pre-transposes/casts: qT [D,SQ], kT/vT [D,SK], wqT/wkT/wvT/wdT [D,D]
    (= W.T, i.e. contraction dim on rows), bd replicated to [128, D] f32.
  - qhT = wq @ q.T   (form: out[o,s] = wqT[i,o].T @ qT[i,s], accum over i)
  - khT likewise; vh = v @ wv.T natural (out[j,o] = vT[i,j].T @ wvT[i,o]),
    stored per head with a ones column appended -> vh_plus [j, 65] so the
    ctx matmul also produces the softmax row-sums.
  - per head: logitsT[j,s] = khT_h.T @ qhT_h (K=64); P_T = exp(logitsT/8)
    (no max subtraction: logits ~ N(0,1), exp is safe in f32);
    ctxT_un[d+,s] = vh_plus.T @ P_T accumulated over j-chunks; row 64 is the
    row-sum; ctxT = ctxT_un * (1/rowsum) via DMA-replicated reciprocal.
  - out = ctxT.T @ wdT + bd (natural layout), DMA out.
  - attn head 0 is written TRANSPOSED (attnT[j,s] = pT0 * 1/rowsum, DVE+DMA
    only, overlapping later heads); the host transposes it back on gather.
"""

import sys
from contextlib import ExitStack

import numpy as np

if "/opt/trn_rl_repo" not in sys.path:
    sys.path.insert(0, "/opt/trn_rl_repo")

import concourse.bass as bass
import concourse.bacc as bacc
import concourse.mybir as mybir
import concourse.tile as tile

F32 = mybir.dt.float32
BF16 = mybir.dt.bfloat16
P = 128
DH = 64  # head depth


def _nslices(total, width=512):
    return [(n0, min(width, total - n0)) for n0 in range(0, total, width)]


def build_nc(SQ=1024, SK=2048, D=768, H=12):
    """Build the per-core Bass program (SPMD; same program on every core)."""
    assert D % P == 0 and SQ % P == 0 and SK % P == 0 and H * DH == D
    NI = D // P    # contraction chunks over model dim
    NSQ = SQ // P  # query-row chunks
    NSK = SK // P  # key-row chunks

    nc = bacc.Bacc("TRN2")

    qT_d = nc.declare_dram_parameter("qT", [D, SQ], BF16, isOutput=False)
    kT_d = nc.declare_dram_parameter("kT", [D, SK], BF16, isOutput=False)
    vT_d = nc.declare_dram_parameter("vT", [D, SK], BF16, isOutput=False)
    wqT_d = nc.declare_dram_parameter("wqT", [D, D], BF16, isOutput=False)
    wkT_d = nc.declare_dram_parameter("wkT", [D, D], BF16, isOutput=False)
    wvT_d = nc.declare_dram_parameter("wvT", [D, D], BF16, isOutput=False)
    wdT_d = nc.declare_dram_parameter("wdT", [D, D], BF16, isOutput=False)
    bdr_d = nc.declare_dram_parameter("bdr", [P, D], F32, isOutput=False)
    out_d = nc.declare_dram_parameter("out", [SQ, D], F32, isOutput=True)
    attnT_d = nc.declare_dram_parameter("attnT", [SK, SQ], F32, isOutput=True)
    invd = nc.dram_tensor("invd", [H, SQ], F32)  # per-head 1/rowsum bounce

    scale = 1.0 / np.sqrt(np.float32(DH))

    with tile.TileContext(nc) as tc, ExitStack() as octx:
        pers = octx.enter_context(tc.tile_pool(name="pers", bufs=1))
        # shared psum rotation (proj, logits, out-proj): 3 x 2 banks; ctx: 2.
        mp = octx.enter_context(tc.tile_pool(name="mpsum", bufs=3, space="PSUM"))
        cp = octx.enter_context(tc.tile_pool(name="cpsum", bufs=1, space="PSUM"))

        qhT = pers.tile([P, NI, SQ], BF16)
        khT = pers.tile([P, NI, SK], BF16)
        vh = pers.tile([P, NSK, H, DH + 1], BF16)  # [.., 64] is the ones col
        ctxT = pers.tile([P, NI, SQ], BF16)
        wdT_s = pers.tile([P, NI, D], BF16)
        bdr_s = pers.tile([P, D], F32)
        pT0 = pers.tile([P, NSK, SQ], BF16)  # head-0 exp(logits^T), kept

        nc.sync.dma_start(wdT_s[:], wdT_d.rearrange("(n p) m -> p n m", p=P))
        nc.sync.dma_start(bdr_s[:], bdr_d[:])
        nc.gpsimd.memset(vh[:, :, :, DH : DH + 1], 1.0)

        scale_f = float(scale)

        # ------- phase 1: load + projections -------
        with tc.tile_pool(name="inw", bufs=1) as inp:
            qT_s = inp.tile([P, NI, SQ], BF16)
            kT_s = inp.tile([P, NI, SK], BF16)
            vT_s = inp.tile([P, NI, SK], BF16)
            wqT_s = inp.tile([P, NI, D], BF16)
            wkT_s = inp.tile([P, NI, D], BF16)
            wvT_s = inp.tile([P, NI, D], BF16)
            # q/wq first so the q-projection can start ASAP
            for ic in range(NI):
                sl = slice(ic * P, (ic + 1) * P)
                nc.sync.dma_start(wqT_s[:, ic, :], wqT_d[sl, :])
                nc.sync.dma_start(qT_s[:, ic, :], qT_d[sl, :])
            for ic in range(NI):
                sl = slice(ic * P, (ic + 1) * P)
                nc.sync.dma_start(wkT_s[:, ic, :], wkT_d[sl, :])
                nc.sync.dma_start(kT_s[:, ic, :], kT_d[sl, :])
            for ic in range(NI):
                sl = slice(ic * P, (ic + 1) * P)
                nc.sync.dma_start(wvT_s[:, ic, :], wvT_d[sl, :])
                nc.sync.dma_start(vT_s[:, ic, :], vT_d[sl, :])

            def proj_qk(oc):
                osl = slice(oc * P, (oc + 1) * P)
                ps = mp.tile([P, SQ], F32, tag="ps", name=f"psq{oc}")
                for ic in range(NI):
                    for n0, w in _nslices(SQ):
                        nc.tensor.matmul(
                            ps[:, n0 : n0 + w],
                            wqT_s[:, ic, osl],
                            qT_s[:, ic, n0 : n0 + w],
                            start=(ic == 0),
                            stop=(ic == NI - 1),
                        )
                nc.vector.tensor_copy(qhT[:, oc, :], ps[:])
                SKB = min(SQ, SK)
                for sh in range(SK // SKB):
                    ssl0 = sh * SKB
                    ps = mp.tile([P, SKB], F32, tag="ps", name=f"psk{oc}_{sh}")
                    for ic in range(NI):
                        for n0, w in _nslices(SKB):
                            nc.tensor.matmul(
                                ps[:, n0 : n0 + w],
                                wkT_s[:, ic, osl],
                                kT_s[:, ic, ssl0 + n0 : ssl0 + n0 + w],
                                start=(ic == 0),
                                stop=(ic == NI - 1),
                            )
                    nc.vector.tensor_copy(khT[:, oc, ssl0 : ssl0 + SKB], ps[:])

            proj_qk(0)

            for oc in range(1, NI):
                proj_qk(oc)

            # vh: out[j-block, o] = sum_ic vT[ic, j].T @ wvT[ic, o]
            for jc in range(NSK):
                jsl = slice(jc * P, (jc + 1) * P)
                ps = mp.tile([P, D], F32, tag="ps")
                for ic in range(NI):
                    for n0, w in _nslices(D):
                        nc.tensor.matmul(
                            ps[:, n0 : n0 + w],
                            vT_s[:, ic, jsl],
                            wvT_s[:, ic, n0 : n0 + w],
                            start=(ic == 0),
                            stop=(ic == NI - 1),
                        )
                nc.vector.tensor_copy(
                    vh[:, jc, :, 0:DH],
                    ps[:].rearrange("p (h d) -> p h d", d=DH),
                )

        # ------- phase 2: attention heads (+ attn-T head-0 output) -------
        with (
            tc.tile_pool(name="ptpool", bufs=12) as ptp,
            tc.tile_pool(name="invp", bufs=2) as invp,
            tc.tile_pool(name="attnp", bufs=2) as atp,
            tc.tile_pool(name="inv0p", bufs=1) as i0p,
        ):
            for h in range(H):
                oc_h = (h * DH) // P
                po = (h * DH) % P
                psl = slice(po, po + DH)
                # logits^T [j, s] per j-chunk, then exp -> P_T
                pts = []
                for jc in range(NSK):
                    pl = mp.tile([P, SQ], F32, tag="ps")
                    for n0, w in _nslices(SQ):
                        nc.tensor.matmul(
                            pl[:, n0 : n0 + w],
                            khT[psl, oc_h, jc * P : (jc + 1) * P],
                            qhT[psl, oc_h, n0 : n0 + w],
                            start=True,
                            stop=True,
                        )
                    if h == 0:
                        pt = pT0[:, jc, :]
                    else:
                        pt_tile = ptp.tile([P, SQ], BF16, tag="pt")
                        pt = pt_tile[:]
                    nc.scalar.activation(
                        pt, pl[:], mybir.ActivationFunctionType.Exp, scale=scale_f
                    )
                    pts.append(pt)
                # ctx^T accumulated over j-chunks; row DH = softmax denom
                cps = cp.tile([DH + 1, SQ], F32, tag="cps")
                for jc in range(NSK):
                    for n0, w in _nslices(SQ):
                        nc.tensor.matmul(
                            cps[:, n0 : n0 + w],
                            vh[:, jc, h, :],
                            pts[jc][:, n0 : n0 + w],
                            start=(jc == 0),
                            stop=(jc == NSK - 1),
                        )
                # drain psum right away; normalization runs off critical path
                cst = invp.tile([DH + 1, SQ], F32, tag="cst")
                nc.vector.tensor_copy(cst[:], cps[:])
                invrow = invp.tile([1, SQ], F32, tag="invrow")
                nc.vector.reciprocal(invrow[:], cst[DH : DH + 1, :])
                nc.sync.dma_start(invd[h : h + 1, :], invrow[:])
                invr = invp.tile([DH, SQ], F32, tag="invr")
                inv_bcast = bass.AP(
                    tensor=invd.tensor if hasattr(invd, "tensor") else invd,
                    offset=h * SQ,
                    ap=[[0, DH], [1, SQ]],
                )
                nc.sync.dma_start(invr[:], inv_bcast)
                nc.vector.tensor_mul(ctxT[psl, oc_h, :], cst[0:DH, :], invr[:])

                if h == 0:
                    # attn-T output: attnT[j, s] = pT0[j, s] * inv0[s]; the
                    # host transposes back. DVE + DMA only, overlaps heads 1+.
                    inv0rep = i0p.tile([P, SQ], F32)
                    inv0_src = bass.AP(
                        tensor=invd.tensor if hasattr(invd, "tensor") else invd,
                        offset=0,
                        ap=[[0, P], [1, SQ]],
                    )
                    nc.sync.dma_start(inv0rep[:], inv0_src)
                    for jc in range(NSK):
                        ast = atp.tile([P, SQ], F32, tag="ast")
                        nc.vector.tensor_mul(ast[:], pT0[:, jc, :], inv0rep[:])
                        nc.sync.dma_start(
                            attnT_d[jc * P : (jc + 1) * P, :], ast[:]
                        )

        # ------- phase 3: output projection -------
        with tc.tile_pool(name="ostage", bufs=2) as osp:
            for sc in range(NSQ):
                ssl = slice(sc * P, (sc + 1) * P)
                ps = mp.tile([P, D], F32, tag="ps")
                for ic in range(NI):
                    for n0, w in _nslices(D):
                        nc.tensor.matmul(
                            ps[:, n0 : n0 + w],
                            ctxT[:, ic, ssl],
                            wdT_s[:, ic, n0 : n0 + w],
                            start=(ic == 0),
                            stop=(ic == NI - 1),
                        )
                ost = osp.tile([P, D], F32, tag="ost")
                nc.vector.tensor_add(ost[:], ps[:], bdr_s[:])
                nc.sync.dma_start(out_d[ssl, :], ost[:])

    nc.finalize()  # Bacc: runs wait-splitting etc. so walrus codegen accepts
    return nc


# ---------------------------------------------------------------------------
# host side
# ---------------------------------------------------------------------------

B, S, D, H = 4, 2048, 768, 12
N_CORES = 8
SQ = S * B // N_CORES  # 1024 query rows per core
SK = S


def _prep_in_maps(q, k, v, wq, wk, wv, wd, bd):
    import ml_dtypes

    bf16 = ml_dtypes.bfloat16
    f32 = np.float32

    def t_bf16(a):  # transpose last two dims, cast to bf16, contiguous
        return np.ascontiguousarray(np.asarray(a, dtype=f32).T.astype(bf16))

    wqT = t_bf16(wq)
    wkT = t_bf16(wk)
    wvT = t_bf16(wv)
    wdT = t_bf16(wd)
    bdr = np.ascontiguousarray(
        np.broadcast_to(np.asarray(bd, dtype=f32), (P, D))
    )
    in_maps = []
    for c in range(N_CORES):
        b, half = c // 2, c % 2
        qs = slice(half * SQ, (half + 1) * SQ)
        in_maps.append(
            {
                "qT": t_bf16(q[b, qs, :]),
                "kT": t_bf16(k[b]),
                "vT": t_bf16(v[b]),
                "wqT": wqT,
                "wkT": wkT,
                "wvT": wvT,
                "wdT": wdT,
                "bdr": bdr,
            }
        )
    return in_maps


def _ensure_ntff_hook():
    """Register the axon NTFF profile hook if the image's antenv lacks it."""
    import types

    try:
        import antenv.axon_hooks as ah
    except ImportError:
        ah = types.ModuleType("antenv.axon_hooks")
        _h = {"hook": None}
        ah.set_axon_ntff_profile_hook = lambda hook: _h.__setitem__("hook", hook)
        ah.get_axon_ntff_profile_hook = lambda: _h["hook"]
        sys.modules["antenv.axon_hooks"] = ah
        import antenv

        antenv.axon_hooks = ah
    if ah.get_axon_ntff_profile_hook() is None:
        try:
            from trn_agent_boot.trn_boot import _ntff_profile_via_ctypes

            ah.set_axon_ntff_profile_hook(
                _ntff_profile_via_ctypes("/opt/axon/libaxon_pjrt.so")
            )
        except Exception as e:  # profiling degrades, run still works
            print("ntff hook setup failed:", e)


def run_spmd(q, k, v, wq, wk, wv, wd, bd, trace=False):
    from concourse.bass_utils import run_bass_kernel_spmd

    if trace:
        _ensure_ntff_hook()

    nc = build_nc(SQ=SQ, SK=SK, D=D, H=H)
    in_maps = _prep_in_maps(q, k, v, wq, wk, wv, wd, bd)
    res = run_bass_kernel_spmd(nc, in_maps, list(range(N_CORES)), trace=trace)

    out = np.empty((B, S, D), dtype=np.float32)
    attn = np.empty((B, S, S), dtype=np.float32)
    for c in range(N_CORES):
        b, half = c // 2, c % 2
        qs = slice(half * SQ, (half + 1) * SQ)
        out[b, qs, :] = res.results[c]["out"]
        attn[b, qs, :] = res.results[c]["attnT"].T
    return (out, attn), res


def kernel(q, k, v, wq, wk, wv, wd, bd):
    (out, attn), _ = run_spmd(q, k, v, wq, wk, wv, wd, bd, trace=False)
    return out, attn


# revision 28
# speedup vs baseline: 1.0301x; 1.0301x over previous
"""Multi-head attention (B=4, S=2048, D=768, H=12) on 8 TRN2 NeuronCores.

Sharding: core c = (batch b = c//2, query-half qh = c%2). Each core computes
the full 12-head attention for its 1024 query rows of its batch (K/V
projections duplicated across the pair of cores sharing a batch), and writes
disjoint slices of both outputs. No collectives.

Per-core kernel (all matmuls bf16 with f32 PSUM accumulate):
  - host pre-transposes/casts: qT [D,SQ], kT/vT [D,SK], wqT/wkT/wvT/wdT [D,D]
    (= W.T, i.e. contraction dim on rows), bd replicated to [128, D] f32.
  - qhT = wq @ q.T   (form: out[o,s] = wqT[i,o].T @ qT[i,s], accum over i)
  - khT likewise; vh = v @ wv.T natural (out[j,o] = vT[i,j].T @ wvT[i,o]),
    stored per head with a ones column appended -> vh_plus [j, 65] so the
    ctx matmul also produces the softmax row-sums.
  - per head: logitsT[j,s] = khT_h.T @ qhT_h (K=64); P_T = exp(logitsT/8)
    (no max subtraction: logits ~ N(0,1), exp is safe in f32);
    ctxT_un[d+,s] = vh_plus.T @ P_T accumulated over j-chunks; row 64 is the
    row-sum; ctxT = ctxT_un * (1/rowsum) via DMA-replicated reciprocal.
  - out = ctxT.T @ wdT + bd (natural layout), DMA out.
  - attn head 0 is written TRANSPOSED (attnT[j,s] = pT0 * 1/rowsum, DVE+DMA
    only, overlapping later heads); the host transposes it back on gather.
"""

import sys
from contextlib import ExitStack

import numpy as np

if "/opt/trn_rl_repo" not in sys.path:
    sys.path.insert(0, "/opt/trn_rl_repo")

import concourse.bass as bass
import concourse.bacc as bacc
import concourse.mybir as mybir
import concourse.tile as tile

F32 = mybir.dt.float32
BF16 = mybir.dt.bfloat16
P = 128
DH = 64  # head depth


def _nslices(total, width=512):
    return [(n0, min(width, total - n0)) for n0 in range(0, total, width)]


def build_nc(SQ=1024, SK=2048, D=768, H=12):
    """Build the per-core Bass program (SPMD; same program on every core)."""
    assert D % P == 0 and SQ % P == 0 and SK % P == 0 and H * DH == D
    NI = D // P    # contraction chunks over model dim
    NSQ = SQ // P  # query-row chunks
    NSK = SK // P  # key-row chunks

    nc = bacc.Bacc("TRN2")

    qT_d = nc.declare_dram_parameter("qT", [D, SQ], BF16, isOutput=False)
    kT_d = nc.declare_dram_parameter("kT", [D, SK], BF16, isOutput=False)
    vT_d = nc.declare_dram_parameter("vT", [D, SK], BF16, isOutput=False)
    wqT_d = nc.declare_dram_parameter("wqT", [D, D], BF16, isOutput=False)
    wkT_d = nc.declare_dram_parameter("wkT", [D, D], BF16, isOutput=False)
    wvT_d = nc.declare_dram_parameter("wvT", [D, D], BF16, isOutput=False)
    wdT_d = nc.declare_dram_parameter("wdT", [D, D], BF16, isOutput=False)
    bdr_d = nc.declare_dram_parameter("bdr", [P, D], F32, isOutput=False)
    out_d = nc.declare_dram_parameter("out", [SQ, D], F32, isOutput=True)
    attnT_d = nc.declare_dram_parameter("attnT", [SK, SQ], F32, isOutput=True)
    invd = nc.dram_tensor("invd", [H, SQ], F32)  # per-head 1/rowsum bounce

    scale = 1.0 / np.sqrt(np.float32(DH))

    with tile.TileContext(nc) as tc, ExitStack() as octx:
        pers = octx.enter_context(tc.tile_pool(name="pers", bufs=1))
        # shared psum rotation (proj, logits, out-proj): 3 x 2 banks; ctx: 2.
        mp = octx.enter_context(tc.tile_pool(name="mpsum", bufs=3, space="PSUM"))
        cp = octx.enter_context(tc.tile_pool(name="cpsum", bufs=1, space="PSUM"))

        qhT = pers.tile([P, NI, SQ], BF16)
        khT = pers.tile([P, NI, SK], BF16)
        vh = pers.tile([P, NSK, H, DH + 1], BF16)  # [.., 64] is the ones col
        ctxT = pers.tile([P, NI, SQ], BF16)
        wdT_s = pers.tile([P, NI, D], BF16)
        bdr_s = pers.tile([P, D], F32)
        pT0 = pers.tile([P, NSK, SQ], BF16)  # head-0 exp(logits^T), kept

        nc.sync.dma_start(wdT_s[:], wdT_d.rearrange("(n p) m -> p n m", p=P))
        nc.sync.dma_start(bdr_s[:], bdr_d[:])
        nc.gpsimd.memset(vh[:, :, :, DH : DH + 1], 1.0)

        scale_f = float(scale)

        # ------- phase 1: load + projections -------
        with tc.tile_pool(name="inw", bufs=1) as inp:
            qT_s = inp.tile([P, NI, SQ], BF16)
            kT_s = inp.tile([P, NI, SK], BF16)
            vT_s = inp.tile([P, NI, SK], BF16)
            wqT_s = inp.tile([P, NI, D], BF16)
            wkT_s = inp.tile([P, NI, D], BF16)
            wvT_s = inp.tile([P, NI, D], BF16)
            # q/wq first so the q-projection can start ASAP
            for ic in range(NI):
                sl = slice(ic * P, (ic + 1) * P)
                nc.sync.dma_start(wqT_s[:, ic, :], wqT_d[sl, :])
                nc.sync.dma_start(qT_s[:, ic, :], qT_d[sl, :])
            for ic in range(NI):
                sl = slice(ic * P, (ic + 1) * P)
                nc.sync.dma_start(wkT_s[:, ic, :], wkT_d[sl, :])
                nc.sync.dma_start(kT_s[:, ic, :], kT_d[sl, :])
            for ic in range(NI):
                sl = slice(ic * P, (ic + 1) * P)
                nc.sync.dma_start(wvT_s[:, ic, :], wvT_d[sl, :])
                nc.sync.dma_start(vT_s[:, ic, :], vT_d[sl, :])

            def proj_qk(oc):
                osl = slice(oc * P, (oc + 1) * P)
                ps = mp.tile([P, SQ], F32, tag="ps", name=f"psq{oc}")
                for ic in range(NI):
                    for n0, w in _nslices(SQ):
                        nc.tensor.matmul(
                            ps[:, n0 : n0 + w],
                            wqT_s[:, ic, osl],
                            qT_s[:, ic, n0 : n0 + w],
                            start=(ic == 0),
                            stop=(ic == NI - 1),
                        )
                nc.vector.tensor_copy(qhT[:, oc, :], ps[:])
                SKB = min(SQ, SK)
                for sh in range(SK // SKB):
                    ssl0 = sh * SKB
                    ps = mp.tile([P, SKB], F32, tag="ps", name=f"psk{oc}_{sh}")
                    for ic in range(NI):
                        for n0, w in _nslices(SKB):
                            nc.tensor.matmul(
                                ps[:, n0 : n0 + w],
                                wkT_s[:, ic, osl],
                                kT_s[:, ic, ssl0 + n0 : ssl0 + n0 + w],
                                start=(ic == 0),
                                stop=(ic == NI - 1),
                            )
                    nc.vector.tensor_copy(khT[:, oc, ssl0 : ssl0 + SKB], ps[:])

            proj_qk(0)

            for oc in range(1, NI):
                proj_qk(oc)

            # vh: out[j-block, o] = sum_ic vT[ic, j].T @ wvT[ic, o]
            for jc in range(NSK):
                jsl = slice(jc * P, (jc + 1) * P)
                ps = mp.tile([P, D], F32, tag="ps")
                for ic in range(NI):
                    for n0, w in _nslices(D):
                        nc.tensor.matmul(
                            ps[:, n0 : n0 + w],
                            vT_s[:, ic, jsl],
                            wvT_s[:, ic, n0 : n0 + w],
                            start=(ic == 0),
                            stop=(ic == NI - 1),
                        )
                nc.vector.tensor_copy(
                    vh[:, jc, :, 0:DH],
                    ps[:].rearrange("p (h d) -> p h d", d=DH),
                )

        # ------- phase 2: attention heads (+ attn-T head-0 output) -------
        with (
            tc.tile_pool(name="ptpool", bufs=16) as ptp,
            tc.tile_pool(name="invp", bufs=3) as invp,
            tc.tile_pool(name="attnp", bufs=2) as atp,
            tc.tile_pool(name="inv0p", bufs=1) as i0p,
        ):
            for h in range(H):
                oc_h = (h * DH) // P
                po = (h * DH) % P
                psl = slice(po, po + DH)
                # logits^T [j, s] per j-chunk, then exp -> P_T
                pts = []
                for jc in range(NSK):
                    pl = mp.tile([P, SQ], F32, tag="ps")
                    for n0, w in _nslices(SQ):
                        nc.tensor.matmul(
                            pl[:, n0 : n0 + w],
                            khT[psl, oc_h, jc * P : (jc + 1) * P],
                            qhT[psl, oc_h, n0 : n0 + w],
                            start=True,
                            stop=True,
                        )
                    if h == 0:
                        pt = pT0[:, jc, :]
                    else:
                        pt_tile = ptp.tile([P, SQ], BF16, tag="pt")
                        pt = pt_tile[:]
                    nc.scalar.activation(
                        pt, pl[:], mybir.ActivationFunctionType.Exp, scale=scale_f
                    )
                    pts.append(pt)
                # ctx^T accumulated over j-chunks; row DH = softmax denom
                cps = cp.tile([DH + 1, SQ], F32, tag="cps")
                for jc in range(NSK):
                    for n0, w in _nslices(SQ):
                        nc.tensor.matmul(
                            cps[:, n0 : n0 + w],
                            vh[:, jc, h, :],
                            pts[jc][:, n0 : n0 + w],
                            start=(jc == 0),
                            stop=(jc == NSK - 1),
                        )
                # drain psum right away; normalization runs off critical path
                cst = invp.tile([DH + 1, SQ], F32, tag="cst")
                nc.vector.tensor_copy(cst[:], cps[:])
                invrow = invp.tile([1, SQ], F32, tag="invrow")
                nc.vector.reciprocal(invrow[:], cst[DH : DH + 1, :])
                nc.sync.dma_start(invd[h : h + 1, :], invrow[:])
                invr = invp.tile([DH, SQ], F32, tag="invr")
                inv_bcast = bass.AP(
                    tensor=invd.tensor if hasattr(invd, "tensor") else invd,
                    offset=h * SQ,
                    ap=[[0, DH], [1, SQ]],
                )
                nc.sync.dma_start(invr[:], inv_bcast)
                nc.vector.tensor_mul(ctxT[psl, oc_h, :], cst[0:DH, :], invr[:])

                if h == 0:
                    # attn-T output: attnT[j, s] = pT0[j, s] * inv0[s]; the
                    # host transposes back. DVE + DMA only, overlaps heads 1+.
                    inv0rep = i0p.tile([P, SQ], F32)
                    inv0_src = bass.AP(
                        tensor=invd.tensor if hasattr(invd, "tensor") else invd,
                        offset=0,
                        ap=[[0, P], [1, SQ]],
                    )
                    nc.sync.dma_start(inv0rep[:], inv0_src)
                    for jc in range(NSK):
                        ast = atp.tile([P, SQ], F32, tag="ast")
                        nc.vector.tensor_mul(ast[:], pT0[:, jc, :], inv0rep[:])
                        nc.sync.dma_start(
                            attnT_d[jc * P : (jc + 1) * P, :], ast[:]
                        )

        # ------- phase 3: output projection -------
        with tc.tile_pool(name="ostage", bufs=2) as osp:
            for sc in range(NSQ):
                ssl = slice(sc * P, (sc + 1) * P)
                ps = mp.tile([P, D], F32, tag="ps")
                for ic in range(NI):
                    for n0, w in _nslices(D):
                        nc.tensor.matmul(
                            ps[:, n0 : n0 + w],
                            ctxT[:, ic, ssl],
                            wdT_s[:, ic, n0 : n0 + w],
                            start=(ic == 0),
                            stop=(ic == NI - 1),
                        )
                ost = osp.tile([P, D], F32, tag="ost")
                nc.vector.tensor_add(ost[:], ps[:], bdr_s[:])
                nc.sync.dma_start(out_d[ssl, :], ost[:])

    nc.finalize()  # Bacc: runs wait-splitting etc. so walrus codegen accepts
    return nc


# ---------------------------------------------------------------------------
# host side
# ---------------------------------------------------------------------------

B, S, D, H = 4, 2048, 768, 12
N_CORES = 8
SQ = S * B // N_CORES  # 1024 query rows per core
SK = S


def _prep_in_maps(q, k, v, wq, wk, wv, wd, bd):
    import ml_dtypes

    bf16 = ml_dtypes.bfloat16
    f32 = np.float32

    def t_bf16(a):  # transpose last two dims, cast to bf16, contiguous
        return np.ascontiguousarray(np.asarray(a, dtype=f32).T.astype(bf16))

    wqT = t_bf16(wq)
    wkT = t_bf16(wk)
    wvT = t_bf16(wv)
    wdT = t_bf16(wd)
    bdr = np.ascontiguousarray(
        np.broadcast_to(np.asarray(bd, dtype=f32), (P, D))
    )
    in_maps = []
    for c in range(N_CORES):
        b, half = c // 2, c % 2
        qs = slice(half * SQ, (half + 1) * SQ)
        in_maps.append(
            {
                "qT": t_bf16(q[b, qs, :]),
                "kT": t_bf16(k[b]),
                "vT": t_bf16(v[b]),
                "wqT": wqT,
                "wkT": wkT,
                "wvT": wvT,
                "wdT": wdT,
                "bdr": bdr,
            }
        )
    return in_maps


def _ensure_ntff_hook():
    """Register the axon NTFF profile hook if the image's antenv lacks it."""
    import types

    try:
        import antenv.axon_hooks as ah
    except ImportError:
        ah = types.ModuleType("antenv.axon_hooks")
        _h = {"hook": None}
        ah.set_axon_ntff_profile_hook = lambda hook: _h.__setitem__("hook", hook)
        ah.get_axon_ntff_profile_hook = lambda: _h["hook"]
        sys.modules["antenv.axon_hooks"] = ah
        import antenv

        antenv.axon_hooks = ah
    if ah.get_axon_ntff_profile_hook() is None:
        try:
            from trn_agent_boot.trn_boot import _ntff_profile_via_ctypes

            ah.set_axon_ntff_profile_hook(
                _ntff_profile_via_ctypes("/opt/axon/libaxon_pjrt.so")
            )
        except Exception as e:  # profiling degrades, run still works
            print("ntff hook setup failed:", e)


def run_spmd(q, k, v, wq, wk, wv, wd, bd, trace=False):
    from concourse.bass_utils import run_bass_kernel_spmd

    if trace:
        _ensure_ntff_hook()

    nc = build_nc(SQ=SQ, SK=SK, D=D, H=H)
    in_maps = _prep_in_maps(q, k, v, wq, wk, wv, wd, bd)
    res = run_bass_kernel_spmd(nc, in_maps, list(range(N_CORES)), trace=trace)

    out = np.empty((B, S, D), dtype=np.float32)
    attn = np.empty((B, S, S), dtype=np.float32)
    for c in range(N_CORES):
        b, half = c // 2, c % 2
        qs = slice(half * SQ, (half + 1) * SQ)
        out[b, qs, :] = res.results[c]["out"]
        attn[b, qs, :] = res.results[c]["attnT"].T
    return (out, attn), res


def kernel(q, k, v, wq, wk, wv, wd, bd):
    (out, attn), _ = run_spmd(q, k, v, wq, wk, wv, wd, bd, trace=False)
    return out, attn


# revision 29
# speedup vs baseline: 1.0663x; 1.0352x over previous
"""Multi-head attention (B=4, S=2048, D=768, H=12) on 8 TRN2 NeuronCores.

Sharding: core c = (batch b = c//2, query-half qh = c%2). Each core computes
the full 12-head attention for its 1024 query rows of its batch (K/V
projections duplicated across the pair of cores sharing a batch), and writes
disjoint slices of both outputs. No collectives.

Per-core kernel (all matmuls bf16 with f32 PSUM accumulate):
  - host pre-transposes/casts: qT [D,SQ], kT/vT [D,SK], wqT/wkT/wvT/wdT [D,D]
    (= W.T, i.e. contraction dim on rows), bd replicated to [128, D] f32.
  - qhT = wq @ q.T   (form: out[o,s] = wqT[i,o].T @ qT[i,s], accum over i)
  - khT likewise; vh = v @ wv.T natural (out[j,o] = vT[i,j].T @ wvT[i,o]),
    stored per head with a ones column appended -> vh_plus [j, 65] so the
    ctx matmul also produces the softmax row-sums.
  - per head: logitsT[j,s] = khT_h.T @ qhT_h (K=64); P_T = exp(logitsT/8)
    (no max subtraction: logits ~ N(0,1), exp is safe in f32);
    ctxT_un[d+,s] = vh_plus.T @ P_T accumulated over j-chunks; row 64 is the
    row-sum; ctxT = ctxT_un * (1/rowsum) via DMA-replicated reciprocal.
  - out = ctxT.T @ wdT + bd (natural layout), DMA out.
  - attn head 0 is written TRANSPOSED (attnT[j,s] = pT0 * 1/rowsum, DVE+DMA
    only, overlapping later heads); the host transposes it back on gather.
"""

import sys
from contextlib import ExitStack

import numpy as np

if "/opt/trn_rl_repo" not in sys.path:
    sys.path.insert(0, "/opt/trn_rl_repo")

import concourse.bass as bass
import concourse.bacc as bacc
import concourse.mybir as mybir
import concourse.tile as tile

F32 = mybir.dt.float32
BF16 = mybir.dt.bfloat16
P = 128
DH = 64  # head depth


def _nslices(total, width=512):
    return [(n0, min(width, total - n0)) for n0 in range(0, total, width)]


def build_nc(SQ=1024, SK=2048, D=768, H=12):
    """Build the per-core Bass program (SPMD; same program on every core)."""
    assert D % P == 0 and SQ % P == 0 and SK % P == 0 and H * DH == D
    NI = D // P    # contraction chunks over model dim
    NSQ = SQ // P  # query-row chunks
    NSK = SK // P  # key-row chunks

    nc = bacc.Bacc("TRN2")

    qT_d = nc.declare_dram_parameter("qT", [D, SQ], BF16, isOutput=False)
    kT_d = nc.declare_dram_parameter("kT", [D, SK], BF16, isOutput=False)
    vT_d = nc.declare_dram_parameter("vT", [D, SK], BF16, isOutput=False)
    wqT_d = nc.declare_dram_parameter("wqT", [D, D], BF16, isOutput=False)
    wkT_d = nc.declare_dram_parameter("wkT", [D, D], BF16, isOutput=False)
    wvT_d = nc.declare_dram_parameter("wvT", [D, D], BF16, isOutput=False)
    wdT_d = nc.declare_dram_parameter("wdT", [D, D], BF16, isOutput=False)
    bdr_d = nc.declare_dram_parameter("bdr", [P, D], F32, isOutput=False)
    out_d = nc.declare_dram_parameter("out", [SQ, D], F32, isOutput=True)
    attnT_d = nc.declare_dram_parameter("attnT", [SK, SQ], F32, isOutput=True)
    invd = nc.dram_tensor("invd", [H, SQ], F32)  # per-head 1/rowsum bounce

    scale = 1.0 / np.sqrt(np.float32(DH))

    with tile.TileContext(nc) as tc, ExitStack() as octx:
        pers = octx.enter_context(tc.tile_pool(name="pers", bufs=1))
        # shared psum rotation (proj, logits, out-proj): 3 x 2 banks; ctx: 2.
        mp = octx.enter_context(tc.tile_pool(name="mpsum", bufs=3, space="PSUM"))
        cp = octx.enter_context(tc.tile_pool(name="cpsum", bufs=1, space="PSUM"))

        qhT = pers.tile([P, NI, SQ], BF16)
        khT = pers.tile([P, NI, SK], BF16)
        vh = pers.tile([P, NSK, H, DH + 1], BF16)  # [.., 64] is the ones col
        ctxT = pers.tile([P, NI, SQ], BF16)
        wdT_s = pers.tile([P, NI, D], BF16)
        bdr_s = pers.tile([P, D], F32)
        pT0 = pers.tile([P, NSK, SQ], BF16)  # head-0 exp(logits^T), kept

        nc.sync.dma_start(wdT_s[:], wdT_d.rearrange("(n p) m -> p n m", p=P))
        nc.sync.dma_start(bdr_s[:], bdr_d[:])
        nc.gpsimd.memset(vh[:, :, :, DH : DH + 1], 1.0)

        scale_f = float(scale)

        # ------- phase 1: load + projections -------
        with tc.tile_pool(name="inw", bufs=1) as inp:
            qT_s = inp.tile([P, NI, SQ], BF16)
            kT_s = inp.tile([P, NI, SK], BF16)
            vT_s = inp.tile([P, NI, SK], BF16)
            wqT_s = inp.tile([P, NI, D], BF16)
            wkT_s = inp.tile([P, NI, D], BF16)
            wvT_s = inp.tile([P, NI, D], BF16)
            # q/wq first so the q-projection can start ASAP
            for ic in range(NI):
                sl = slice(ic * P, (ic + 1) * P)
                nc.sync.dma_start(wqT_s[:, ic, :], wqT_d[sl, :])
                nc.sync.dma_start(qT_s[:, ic, :], qT_d[sl, :])
            for ic in range(NI):
                sl = slice(ic * P, (ic + 1) * P)
                nc.sync.dma_start(wkT_s[:, ic, :], wkT_d[sl, :])
                nc.sync.dma_start(kT_s[:, ic, :], kT_d[sl, :])
            for ic in range(NI):
                sl = slice(ic * P, (ic + 1) * P)
                nc.sync.dma_start(wvT_s[:, ic, :], wvT_d[sl, :])
                nc.sync.dma_start(vT_s[:, ic, :], vT_d[sl, :])

            def proj_qk(oc):
                osl = slice(oc * P, (oc + 1) * P)
                ps = mp.tile([P, SQ], F32, tag="ps", name=f"psq{oc}")
                for ic in range(NI):
                    for n0, w in _nslices(SQ):
                        nc.tensor.matmul(
                            ps[:, n0 : n0 + w],
                            wqT_s[:, ic, osl],
                            qT_s[:, ic, n0 : n0 + w],
                            start=(ic == 0),
                            stop=(ic == NI - 1),
                        )
                nc.vector.tensor_copy(qhT[:, oc, :], ps[:])
                SKB = min(SQ, SK)
                for sh in range(SK // SKB):
                    ssl0 = sh * SKB
                    ps = mp.tile([P, SKB], F32, tag="ps", name=f"psk{oc}_{sh}")
                    for ic in range(NI):
                        for n0, w in _nslices(SKB):
                            nc.tensor.matmul(
                                ps[:, n0 : n0 + w],
                                wkT_s[:, ic, osl],
                                kT_s[:, ic, ssl0 + n0 : ssl0 + n0 + w],
                                start=(ic == 0),
                                stop=(ic == NI - 1),
                            )
                    nc.vector.tensor_copy(khT[:, oc, ssl0 : ssl0 + SKB], ps[:])

            proj_qk(0)

            for oc in range(1, NI):
                proj_qk(oc)

            # vh: out[j-block, o] = sum_ic vT[ic, j].T @ wvT[ic, o]
            for jc in range(NSK):
                jsl = slice(jc * P, (jc + 1) * P)
                ps = mp.tile([P, D], F32, tag="ps")
                for ic in range(NI):
                    for n0, w in _nslices(D):
                        nc.tensor.matmul(
                            ps[:, n0 : n0 + w],
                            vT_s[:, ic, jsl],
                            wvT_s[:, ic, n0 : n0 + w],
                            start=(ic == 0),
                            stop=(ic == NI - 1),
                        )
                nc.vector.tensor_copy(
                    vh[:, jc, :, 0:DH],
                    ps[:].rearrange("p (h d) -> p h d", d=DH),
                )

        # ------- phase 2: attention heads (+ attn-T head-0 output) -------
        with (
            tc.tile_pool(name="ptpool", bufs=16) as ptp,
            tc.tile_pool(name="invp", bufs=3) as invp,
            tc.tile_pool(name="attnp", bufs=3) as atp,
            tc.tile_pool(name="inv0p", bufs=1) as i0p,
        ):
            for h in range(H):
                oc_h = (h * DH) // P
                po = (h * DH) % P
                psl = slice(po, po + DH)
                # logits^T [j, s] per j-chunk, then exp -> P_T
                pts = []
                for jc in range(NSK):
                    pl = mp.tile([P, SQ], F32, tag="ps")
                    for n0, w in _nslices(SQ):
                        nc.tensor.matmul(
                            pl[:, n0 : n0 + w],
                            khT[psl, oc_h, jc * P : (jc + 1) * P],
                            qhT[psl, oc_h, n0 : n0 + w],
                            start=True,
                            stop=True,
                        )
                    if h == 0:
                        pt = pT0[:, jc, :]
                    else:
                        pt_tile = ptp.tile([P, SQ], BF16, tag="pt")
                        pt = pt_tile[:]
                    nc.scalar.activation(
                        pt, pl[:], mybir.ActivationFunctionType.Exp, scale=scale_f
                    )
                    pts.append(pt)
                # ctx^T accumulated over j-chunks; row DH = softmax denom
                cps = cp.tile([DH + 1, SQ], F32, tag="cps")
                for jc in range(NSK):
                    for n0, w in _nslices(SQ):
                        nc.tensor.matmul(
                            cps[:, n0 : n0 + w],
                            vh[:, jc, h, :],
                            pts[jc][:, n0 : n0 + w],
                            start=(jc == 0),
                            stop=(jc == NSK - 1),
                        )
                # drain psum right away; normalization runs off critical path
                cst = invp.tile([DH + 1, SQ], F32, tag="cst")
                nc.vector.tensor_copy(cst[:], cps[:])
                invrow = invp.tile([1, SQ], F32, tag="invrow")
                nc.vector.reciprocal(invrow[:], cst[DH : DH + 1, :])
                nc.sync.dma_start(invd[h : h + 1, :], invrow[:])
                invr = invp.tile([DH, SQ], F32, tag="invr")
                inv_bcast = bass.AP(
                    tensor=invd.tensor if hasattr(invd, "tensor") else invd,
                    offset=h * SQ,
                    ap=[[0, DH], [1, SQ]],
                )
                nc.sync.dma_start(invr[:], inv_bcast)
                nc.vector.tensor_mul(ctxT[psl, oc_h, :], cst[0:DH, :], invr[:])

                if h == 0:
                    # attn-T output: attnT[j, s] = pT0[j, s] * inv0[s]; the
                    # host transposes back. DVE + DMA only, overlaps heads 1+.
                    inv0rep = i0p.tile([P, SQ], F32)
                    inv0_src = bass.AP(
                        tensor=invd.tensor if hasattr(invd, "tensor") else invd,
                        offset=0,
                        ap=[[0, P], [1, SQ]],
                    )
                    nc.sync.dma_start(inv0rep[:], inv0_src)
                    for jc in range(NSK):
                        ast = atp.tile([P, SQ], F32, tag="ast")
                        # GPSIMD is otherwise idle; keeps DVE free for the
                        # ctx-drain -> pt-slot -> exp chain of later heads
                        nc.gpsimd.tensor_mul(ast[:], pT0[:, jc, :], inv0rep[:])
                        nc.sync.dma_start(
                            attnT_d[jc * P : (jc + 1) * P, :], ast[:]
                        )

        # ------- phase 3: output projection -------
        with tc.tile_pool(name="ostage", bufs=2) as osp:
            for sc in range(NSQ):
                ssl = slice(sc * P, (sc + 1) * P)
                ps = mp.tile([P, D], F32, tag="ps")
                for ic in range(NI):
                    for n0, w in _nslices(D):
                        nc.tensor.matmul(
                            ps[:, n0 : n0 + w],
                            ctxT[:, ic, ssl],
                            wdT_s[:, ic, n0 : n0 + w],
                            start=(ic == 0),
                            stop=(ic == NI - 1),
                        )
                ost = osp.tile([P, D], F32, tag="ost")
                nc.vector.tensor_add(ost[:], ps[:], bdr_s[:])
                nc.sync.dma_start(out_d[ssl, :], ost[:])

    nc.finalize()  # Bacc: runs wait-splitting etc. so walrus codegen accepts
    return nc


# ---------------------------------------------------------------------------
# host side
# ---------------------------------------------------------------------------

B, S, D, H = 4, 2048, 768, 12
N_CORES = 8
SQ = S * B // N_CORES  # 1024 query rows per core
SK = S


def _prep_in_maps(q, k, v, wq, wk, wv, wd, bd):
    import ml_dtypes

    bf16 = ml_dtypes.bfloat16
    f32 = np.float32

    def t_bf16(a):  # transpose last two dims, cast to bf16, contiguous
        return np.ascontiguousarray(np.asarray(a, dtype=f32).T.astype(bf16))

    wqT = t_bf16(wq)
    wkT = t_bf16(wk)
    wvT = t_bf16(wv)
    wdT = t_bf16(wd)
    bdr = np.ascontiguousarray(
        np.broadcast_to(np.asarray(bd, dtype=f32), (P, D))
    )
    in_maps = []
    for c in range(N_CORES):
        b, half = c // 2, c % 2
        qs = slice(half * SQ, (half + 1) * SQ)
        in_maps.append(
            {
                "qT": t_bf16(q[b, qs, :]),
                "kT": t_bf16(k[b]),
                "vT": t_bf16(v[b]),
                "wqT": wqT,
                "wkT": wkT,
                "wvT": wvT,
                "wdT": wdT,
                "bdr": bdr,
            }
        )
    return in_maps


def _ensure_ntff_hook():
    """Register the axon NTFF profile hook if the image's antenv lacks it."""
    import types

    try:
        import antenv.axon_hooks as ah
    except ImportError:
        ah = types.ModuleType("antenv.axon_hooks")
        _h = {"hook": None}
        ah.set_axon_ntff_profile_hook = lambda hook: _h.__setitem__("hook", hook)
        ah.get_axon_ntff_profile_hook = lambda: _h["hook"]
        sys.modules["antenv.axon_hooks"] = ah
        import antenv

        antenv.axon_hooks = ah
    if ah.get_axon_ntff_profile_hook() is None:
        try:
            from trn_agent_boot.trn_boot import _ntff_profile_via_ctypes

            ah.set_axon_ntff_profile_hook(
                _ntff_profile_via_ctypes("/opt/axon/libaxon_pjrt.so")
            )
        except Exception as e:  # profiling degrades, run still works
            print("ntff hook setup failed:", e)


def run_spmd(q, k, v, wq, wk, wv, wd, bd, trace=False):
    from concourse.bass_utils import run_bass_kernel_spmd

    if trace:
        _ensure_ntff_hook()

    nc = build_nc(SQ=SQ, SK=SK, D=D, H=H)
    in_maps = _prep_in_maps(q, k, v, wq, wk, wv, wd, bd)
    res = run_bass_kernel_spmd(nc, in_maps, list(range(N_CORES)), trace=trace)

    out = np.empty((B, S, D), dtype=np.float32)
    attn = np.empty((B, S, S), dtype=np.float32)
    for c in range(N_CORES):
        b, half = c // 2, c % 2
        qs = slice(half * SQ, (half + 1) * SQ)
        out[b, qs, :] = res.results[c]["out"]
        attn[b, qs, :] = res.results[c]["attnT"].T
    return (out, attn), res


def kernel(q, k, v, wq, wk, wv, wd, bd):
    (out, attn), _ = run_spmd(q, k, v, wq, wk, wv, wd, bd, trace=False)
    return out, attn


# revision 30
# speedup vs baseline: 1.1065x; 1.0377x over previous
"""Multi-head attention (B=4, S=2048, D=768, H=12) on 8 TRN2 NeuronCores.

Sharding: core c = (batch b = c//2, query-half qh = c%2). Each core computes
the full 12-head attention for its 1024 query rows of its batch (K/V
projections duplicated across the pair of cores sharing a batch), and writes
disjoint slices of both outputs. No collectives.

Per-core kernel (all matmuls bf16 with f32 PSUM accumulate):
  - host pre-transposes/casts: qT [D,SQ], kT/vT [D,SK], wqT/wkT/wvT/wdT [D,D]
    (= W.T, i.e. contraction dim on rows), bd replicated to [128, D] f32.
  - qhT = wq @ q.T   (form: out[o,s] = wqT[i,o].T @ qT[i,s], accum over i)
  - khT likewise; vh = v @ wv.T natural (out[j,o] = vT[i,j].T @ wvT[i,o]),
    stored per head with a ones column appended -> vh_plus [j, 65] so the
    ctx matmul also produces the softmax row-sums.
  - per head: logitsT[j,s] = khT_h.T @ qhT_h (K=64); P_T = exp(logitsT/8)
    (no max subtraction: logits ~ N(0,1), exp is safe in f32);
    ctxT_un[d+,s] = vh_plus.T @ P_T accumulated over j-chunks; row 64 is the
    row-sum; ctxT = ctxT_un * (1/rowsum) via DMA-replicated reciprocal.
  - out = ctxT.T @ wdT + bd (natural layout), DMA out.
  - attn head 0 is written TRANSPOSED (attnT[j,s] = pT0 * 1/rowsum, DVE+DMA
    only, overlapping later heads); the host transposes it back on gather.
"""

import sys
from contextlib import ExitStack

import numpy as np

if "/opt/trn_rl_repo" not in sys.path:
    sys.path.insert(0, "/opt/trn_rl_repo")

import concourse.bass as bass
import concourse.bacc as bacc
import concourse.mybir as mybir
import concourse.tile as tile

F32 = mybir.dt.float32
BF16 = mybir.dt.bfloat16
P = 128
DH = 64  # head depth


def _nslices(total, width=512):
    return [(n0, min(width, total - n0)) for n0 in range(0, total, width)]


def build_nc(SQ=1024, SK=2048, D=768, H=12):
    """Build the per-core Bass program (SPMD; same program on every core)."""
    assert D % P == 0 and SQ % P == 0 and SK % P == 0 and H * DH == D
    NI = D // P    # contraction chunks over model dim
    NSQ = SQ // P  # query-row chunks
    NSK = SK // P  # key-row chunks

    nc = bacc.Bacc("TRN2")

    qT_d = nc.declare_dram_parameter("qT", [D, SQ], BF16, isOutput=False)
    kT_d = nc.declare_dram_parameter("kT", [D, SK], BF16, isOutput=False)
    vT_d = nc.declare_dram_parameter("vT", [D, SK], BF16, isOutput=False)
    wqT_d = nc.declare_dram_parameter("wqT", [D, D], BF16, isOutput=False)
    wkT_d = nc.declare_dram_parameter("wkT", [D, D], BF16, isOutput=False)
    wvT_d = nc.declare_dram_parameter("wvT", [D, D], BF16, isOutput=False)
    wdT_d = nc.declare_dram_parameter("wdT", [D, D], BF16, isOutput=False)
    bdr_d = nc.declare_dram_parameter("bdr", [P, D], F32, isOutput=False)
    out_d = nc.declare_dram_parameter("out", [SQ, D], F32, isOutput=True)
    attnT_d = nc.declare_dram_parameter("attnT", [SK, SQ], F32, isOutput=True)
    invd = nc.dram_tensor("invd", [H, SQ], F32)  # per-head 1/rowsum bounce

    scale = 1.0 / np.sqrt(np.float32(DH))

    with tile.TileContext(nc) as tc, ExitStack() as octx:
        pers = octx.enter_context(tc.tile_pool(name="pers", bufs=1))
        # shared psum rotation (proj, logits, out-proj): 3 x 2 banks; ctx: 2.
        mp = octx.enter_context(tc.tile_pool(name="mpsum", bufs=3, space="PSUM"))
        cp = octx.enter_context(tc.tile_pool(name="cpsum", bufs=1, space="PSUM"))

        qhT = pers.tile([P, NI, SQ], BF16)
        khT = pers.tile([P, NI, SK], BF16)
        vh = pers.tile([P, NSK, H, DH + 1], BF16)  # [.., 64] is the ones col
        ctxT = pers.tile([P, NI, SQ], BF16)
        wdT_s = pers.tile([P, NI, D], BF16)
        bdr_s = pers.tile([P, D], F32)
        pT0 = pers.tile([P, NSK, SQ], BF16)  # head-0 exp(logits^T), kept

        nc.sync.dma_start(wdT_s[:], wdT_d.rearrange("(n p) m -> p n m", p=P))
        nc.sync.dma_start(bdr_s[:], bdr_d[:])
        nc.gpsimd.memset(vh[:, :, :, DH : DH + 1], 1.0)

        scale_f = float(scale)

        # ------- phase 1: load + projections -------
        with tc.tile_pool(name="inw", bufs=1) as inp:
            qT_s = inp.tile([P, NI, SQ], BF16)
            kT_s = inp.tile([P, NI, SK], BF16)
            vT_s = inp.tile([P, NI, SK], BF16)
            wqT_s = inp.tile([P, NI, D], BF16)
            wkT_s = inp.tile([P, NI, D], BF16)
            wvT_s = inp.tile([P, NI, D], BF16)
            # q/wq first so the q-projection can start ASAP
            for ic in range(NI):
                sl = slice(ic * P, (ic + 1) * P)
                nc.sync.dma_start(wqT_s[:, ic, :], wqT_d[sl, :])
                nc.sync.dma_start(qT_s[:, ic, :], qT_d[sl, :])
            for ic in range(NI):
                sl = slice(ic * P, (ic + 1) * P)
                nc.sync.dma_start(wkT_s[:, ic, :], wkT_d[sl, :])
                nc.sync.dma_start(kT_s[:, ic, :], kT_d[sl, :])
            for ic in range(NI):
                sl = slice(ic * P, (ic + 1) * P)
                nc.sync.dma_start(wvT_s[:, ic, :], wvT_d[sl, :])
                nc.sync.dma_start(vT_s[:, ic, :], vT_d[sl, :])

            def proj_qk(oc):
                osl = slice(oc * P, (oc + 1) * P)
                ps = mp.tile([P, SQ], F32, tag="ps", name=f"psq{oc}")
                for ic in range(NI):
                    for n0, w in _nslices(SQ):
                        nc.tensor.matmul(
                            ps[:, n0 : n0 + w],
                            wqT_s[:, ic, osl],
                            qT_s[:, ic, n0 : n0 + w],
                            start=(ic == 0),
                            stop=(ic == NI - 1),
                        )
                nc.vector.tensor_copy(qhT[:, oc, :], ps[:])
                SKB = min(SQ, SK)
                for sh in range(SK // SKB):
                    ssl0 = sh * SKB
                    ps = mp.tile([P, SKB], F32, tag="ps", name=f"psk{oc}_{sh}")
                    for ic in range(NI):
                        for n0, w in _nslices(SKB):
                            nc.tensor.matmul(
                                ps[:, n0 : n0 + w],
                                wkT_s[:, ic, osl],
                                kT_s[:, ic, ssl0 + n0 : ssl0 + n0 + w],
                                start=(ic == 0),
                                stop=(ic == NI - 1),
                            )
                    nc.vector.tensor_copy(khT[:, oc, ssl0 : ssl0 + SKB], ps[:])

            proj_qk(0)

            for oc in range(1, NI):
                proj_qk(oc)

            # vh: out[j-block, o] = sum_ic vT[ic, j].T @ wvT[ic, o]
            for jc in range(NSK):
                jsl = slice(jc * P, (jc + 1) * P)
                ps = mp.tile([P, D], F32, tag="ps")
                for ic in range(NI):
                    for n0, w in _nslices(D):
                        nc.tensor.matmul(
                            ps[:, n0 : n0 + w],
                            vT_s[:, ic, jsl],
                            wvT_s[:, ic, n0 : n0 + w],
                            start=(ic == 0),
                            stop=(ic == NI - 1),
                        )
                nc.vector.tensor_copy(
                    vh[:, jc, :, 0:DH],
                    ps[:].rearrange("p (h d) -> p h d", d=DH),
                )

        # ------- phase 2: attention heads (+ attn-T head-0 output) -------
        with (
            tc.tile_pool(name="ptpool", bufs=16) as ptp,
            tc.tile_pool(name="invp", bufs=3) as invp,
            tc.tile_pool(name="attnp", bufs=3) as atp,
            tc.tile_pool(name="inv0p", bufs=1) as i0p,
        ):
            for h in range(H):
                oc_h = (h * DH) // P
                po = (h * DH) % P
                psl = slice(po, po + DH)
                # logits^T [j, s] per j-chunk, then exp -> P_T
                pts = []
                for jc in range(NSK):
                    pl = mp.tile([P, SQ], F32, tag="ps")
                    for n0, w in _nslices(SQ):
                        nc.tensor.matmul(
                            pl[:, n0 : n0 + w],
                            khT[psl, oc_h, jc * P : (jc + 1) * P],
                            qhT[psl, oc_h, n0 : n0 + w],
                            start=True,
                            stop=True,
                        )
                    if h == 0:
                        pt = pT0[:, jc, :]
                    else:
                        pt_tile = ptp.tile([P, SQ], BF16, tag="pt")
                        pt = pt_tile[:]
                    nc.scalar.activation(
                        pt, pl[:], mybir.ActivationFunctionType.Exp, scale=scale_f
                    )
                    pts.append(pt)
                # ctx^T accumulated over j-chunks; row DH = softmax denom
                cps = cp.tile([DH + 1, SQ], F32, tag="cps")
                for jc in range(NSK):
                    for n0, w in _nslices(SQ):
                        nc.tensor.matmul(
                            cps[:, n0 : n0 + w],
                            vh[:, jc, h, :],
                            pts[jc][:, n0 : n0 + w],
                            start=(jc == 0),
                            stop=(jc == NSK - 1),
                        )
                # drain psum right away; normalization runs off critical path
                cst = invp.tile([DH + 1, SQ], F32, tag="cst")
                nc.vector.tensor_copy(cst[:], cps[:])
                invrow = invp.tile([1, SQ], F32, tag="invrow")
                nc.vector.reciprocal(invrow[:], cst[DH : DH + 1, :])
                nc.gpsimd.dma_start(invd[h : h + 1, :], invrow[:])
                invr = invp.tile([DH, SQ], F32, tag="invr")
                inv_bcast = bass.AP(
                    tensor=invd.tensor if hasattr(invd, "tensor") else invd,
                    offset=h * SQ,
                    ap=[[0, DH], [1, SQ]],
                )
                nc.gpsimd.dma_start(invr[:], inv_bcast)
                nc.gpsimd.tensor_mul(ctxT[psl, oc_h, :], cst[0:DH, :], invr[:])

                if h == 0:
                    # attn-T output: attnT[j, s] = pT0[j, s] * inv0[s]; the
                    # host transposes back. DVE + DMA only, overlaps heads 1+.
                    inv0rep = i0p.tile([P, SQ], F32)
                    inv0_src = bass.AP(
                        tensor=invd.tensor if hasattr(invd, "tensor") else invd,
                        offset=0,
                        ap=[[0, P], [1, SQ]],
                    )
                    nc.sync.dma_start(inv0rep[:], inv0_src)
                    for jc in range(NSK):
                        ast = atp.tile([P, SQ], F32, tag="ast")
                        # GPSIMD is otherwise idle; keeps DVE free for the
                        # ctx-drain -> pt-slot -> exp chain of later heads
                        nc.gpsimd.tensor_mul(ast[:], pT0[:, jc, :], inv0rep[:])
                        nc.sync.dma_start(
                            attnT_d[jc * P : (jc + 1) * P, :], ast[:]
                        )

        # ------- phase 3: output projection -------
        with tc.tile_pool(name="ostage", bufs=2) as osp:
            for sc in range(NSQ):
                ssl = slice(sc * P, (sc + 1) * P)
                ps = mp.tile([P, D], F32, tag="ps")
                for ic in range(NI):
                    for n0, w in _nslices(D):
                        nc.tensor.matmul(
                            ps[:, n0 : n0 + w],
                            ctxT[:, ic, ssl],
                            wdT_s[:, ic, n0 : n0 + w],
                            start=(ic == 0),
                            stop=(ic == NI - 1),
                        )
                ost = osp.tile([P, D], F32, tag="ost")
                nc.vector.tensor_add(ost[:], ps[:], bdr_s[:])
                nc.sync.dma_start(out_d[ssl, :], ost[:])

    nc.finalize()  # Bacc: runs wait-splitting etc. so walrus codegen accepts
    return nc


# ---------------------------------------------------------------------------
# host side
# ---------------------------------------------------------------------------

B, S, D, H = 4, 2048, 768, 12
N_CORES = 8
SQ = S * B // N_CORES  # 1024 query rows per core
SK = S


def _prep_in_maps(q, k, v, wq, wk, wv, wd, bd):
    import ml_dtypes

    bf16 = ml_dtypes.bfloat16
    f32 = np.float32

    def t_bf16(a):  # transpose last two dims, cast to bf16, contiguous
        return np.ascontiguousarray(np.asarray(a, dtype=f32).T.astype(bf16))

    wqT = t_bf16(wq)
    wkT = t_bf16(wk)
    wvT = t_bf16(wv)
    wdT = t_bf16(wd)
    bdr = np.ascontiguousarray(
        np.broadcast_to(np.asarray(bd, dtype=f32), (P, D))
    )
    in_maps = []
    for c in range(N_CORES):
        b, half = c // 2, c % 2
        qs = slice(half * SQ, (half + 1) * SQ)
        in_maps.append(
            {
                "qT": t_bf16(q[b, qs, :]),
                "kT": t_bf16(k[b]),
                "vT": t_bf16(v[b]),
                "wqT": wqT,
                "wkT": wkT,
                "wvT": wvT,
                "wdT": wdT,
                "bdr": bdr,
            }
        )
    return in_maps


def _ensure_ntff_hook():
    """Register the axon NTFF profile hook if the image's antenv lacks it."""
    import types

    try:
        import antenv.axon_hooks as ah
    except ImportError:
        ah = types.ModuleType("antenv.axon_hooks")
        _h = {"hook": None}
        ah.set_axon_ntff_profile_hook = lambda hook: _h.__setitem__("hook", hook)
        ah.get_axon_ntff_profile_hook = lambda: _h["hook"]
        sys.modules["antenv.axon_hooks"] = ah
        import antenv

        antenv.axon_hooks = ah
    if ah.get_axon_ntff_profile_hook() is None:
        try:
            from trn_agent_boot.trn_boot import _ntff_profile_via_ctypes

            ah.set_axon_ntff_profile_hook(
                _ntff_profile_via_ctypes("/opt/axon/libaxon_pjrt.so")
            )
        except Exception as e:  # profiling degrades, run still works
            print("ntff hook setup failed:", e)


def run_spmd(q, k, v, wq, wk, wv, wd, bd, trace=False):
    from concourse.bass_utils import run_bass_kernel_spmd

    if trace:
        _ensure_ntff_hook()

    nc = build_nc(SQ=SQ, SK=SK, D=D, H=H)
    in_maps = _prep_in_maps(q, k, v, wq, wk, wv, wd, bd)
    res = run_bass_kernel_spmd(nc, in_maps, list(range(N_CORES)), trace=trace)

    out = np.empty((B, S, D), dtype=np.float32)
    attn = np.empty((B, S, S), dtype=np.float32)
    for c in range(N_CORES):
        b, half = c // 2, c % 2
        qs = slice(half * SQ, (half + 1) * SQ)
        out[b, qs, :] = res.results[c]["out"]
        attn[b, qs, :] = res.results[c]["attnT"].T
    return (out, attn), res


def kernel(q, k, v, wq, wk, wv, wd, bd):
    (out, attn), _ = run_spmd(q, k, v, wq, wk, wv, wd, bd, trace=False)
    return out, attn
